# revision 7
# baseline (speedup 1.0000x reference)
"""Trainium2 Bass kernel for a bidirectional-Mamba decoder layer.

Sharding: data-parallel over batch, one sequence per NeuronCore (B=8, 8 cores).
Layout: transposed throughout (features on partitions, time on free dim).

Schedule (per core):
  A: fwd front-end (in_proj/conv/xproj/dt)           - tensor/scalar
  B: fwd scan block (vector+gpsimd+scalar) overlapped with rev front-end
  C: rev scan block overlapped with fwd+rev out_proj  - vector-bound
  D: LN1 + FFN + LN2 tail                             - tensor/scalar

Engine split in scan blocks: vector = merged 2048-wide segmented scans + hC
mult; gpsimd = du mult, dBu mult (broadcast-AP), B/C partition-broadcasts;
scalar = dA exponentials; tensor = y-reduce identity matmuls + out_proj.
"""
import sys
sys.path.insert(0, "/opt/trn_rl_repo")

import functools
import numpy as np

import concourse.bass as bass
import concourse.mybir as mybir
import concourse.tile as tile
from concourse import bacc
from concourse.bass import ts
from concourse.bass_utils import run_bass_kernel_spmd
from concourse.masks import make_identity

# Restrict activation-table choice so the table-load pass doesn't ping-pong
# between equivalent tables. Index order must be preserved (the emitted
# act_func_set_id is the index into act_info.json), so unwanted tables are
# emptied in place rather than removed.
import concourse.hw_specs as _hw_specs
_KEEP_TABLES = {"natural_log_exp_and_others", "sqrt_and_others",
                "gelu_and_others", "silu_and_others"}
_orig_get_tables = _hw_specs.get_activation_tables
_tab_cache = {}


def _filtered_tables(arch):
    if arch not in _tab_cache:
        t = _orig_get_tables(arch)
        _tab_cache[arch] = {k: (v if k in _KEEP_TABLES else set()) for k, v in t.items()}
    return _tab_cache[arch]


_hw_specs.get_activation_tables = _filtered_tables
import concourse.bacc as _bacc_mod
_bacc_mod.get_activation_tables = _filtered_tables

FP32 = mybir.dt.float32
BF16 = mybir.dt.bfloat16
AOP = mybir.AluOpType
AF = mybir.ActivationFunctionType

DM, DI, DS, DTR, DFF, L = 512, 1024, 16, 32, 2048, 512
NDM, NDI, NFF = DM // 128, DI // 128, DFF // 128   # 4, 8, 16
NB = 8   # batch == cores
NQ = 4   # n-states per merged scan quad
QW = NQ * L  # 2048

W_SHAPES = {}
for p in ("f", "r"):
    W_SHAPES.update({
        p + "_in_w": (2 * DI, DM), p + "_conv_w": (DI, 4), p + "_conv_b": (DI,),
        p + "_xproj_w": (DTR + 2 * DS, DI), p + "_dt_w": (DI, DTR), p + "_dt_b": (DI,),
        p + "_A_log": (DI, DS), p + "_D": (DI,), p + "_out_w": (DM, DI),
    })
W_SHAPES.update({
    "conv1_w": (DFF, DM), "conv1_b": (DFF,), "conv2_w": (DM, DFF), "conv2_b": (DM,),
    "ln1_g": (DM,), "ln1_b": (DM,), "ln2_g": (DM,), "ln2_b": (DM,),
})
T_SHAPES = {}
for p in ("f", "r"):
    T_SHAPES.update({
        p + "_in_wT": (DM, 2 * DI), p + "_xproj_wT": (DI, DTR + 2 * DS),
        p + "_dt_wT": (DTR, DI), p + "_out_wT": (DI, DM),
    })
T_SHAPES.update({"conv1_wT": (DM, DFF), "conv2_wT": (DFF, DM)})
T_SOURCES = {n: n[:-1] for n in T_SHAPES}  # strip trailing T -> source name


def _dcols(ap_1d):
    """(1024,) DRAM tensor -> [128, 8] AP (d-tile index on free dim)."""
    return ap_1d.rearrange("(o p) -> p o", p=128)


def _mcols(ap_1d, n):
    """(n*128,) DRAM tensor -> [128, n] AP."""
    return ap_1d.rearrange("(o p) -> p o", p=128)


class P:
    """Pool/shared-state holder."""
    pass


def _frontend_units(tc, pp, ins, xTb, pfx, rev, use_silu_table):
    """Generator: emits one front-end unit per yield. Fills pp.st[pfx]."""
    nc = tc.nc
    st = pp.st[pfx] = P()
    d = lambda name: ins[pfx + "_" + name]

    # ---- in_proj: xz^T [2048, 512] = in_w^T-tiles @ x^T
    in_wT = ins[pfx + "_in_wT"]
    st.u_pad = [None] * NDI
    st.silu_z = [None] * NDI
    for mi in range(2 * NDI):
        ps = pp.pwork.tile([128, L], FP32, name=f"ps_in_{pfx}_{mi}", tag="work")
        for ki in range(NDM):
            w = pp.wpool.tile([128, 128], BF16, name=f"w_in_{pfx}_{mi}_{ki}", tag="wk", bufs=6)
            nc.sync.dma_start(w, in_wT[ts(ki, 128), ts(mi, 128)])
            nc.tensor.matmul(ps, w, xTb[ki], start=(ki == 0), stop=(ki == NDM - 1))
        if mi < NDI:
            up = pp.bpool.tile([128, L + 6], BF16, name=f"u_pad_{pfx}_{mi}",
                               tag=f"u_pad{mi}", bufs=1)
            nc.vector.memset(up[:, 0:3], 0)
            nc.vector.memset(up[:, L + 3:L + 6], 0)
            nc.scalar.activation(up[:, 3:L + 3], ps, AF.Copy)
            st.u_pad[mi] = up
        else:
            zi = mi - NDI
            sz = pp.bpool.tile([128, L], BF16, name=f"silu_z_{pfx}_{zi}",
                               tag=f"silu_z_{pfx}{zi}", bufs=1)
            if use_silu_table:
                nc.scalar.activation(sz, ps, AF.Silu)
            else:
                # exp/ln chain keeps the scalar engine in natural_log_exp
                zx = pp.spool.tile([128, L], BF16, name=f"zx_{pfx}_{zi}", tag="fe_t1", bufs=1)
                nc.scalar.activation(zx, ps, AF.Copy)
                e1 = pp.spool.tile([128, L], BF16, name=f"e1z_{pfx}_{zi}", tag="fe_t2", bufs=1)
                nc.scalar.activation(e1, ps, AF.Exp, scale=-1.0)
                sp = pp.spool.tile([128, L], BF16, name=f"spz_{pfx}_{zi}", tag="fe_t3", bufs=1)
                nc.scalar.activation(sp, e1, AF.Ln, bias=pp.ones_col)
                e2 = pp.spool.tile([128, L], BF16, name=f"e2z_{pfx}_{zi}", tag="fe_t2", bufs=1)
                nc.scalar.activation(e2, sp, AF.Exp, scale=-1.0)
                nc.gpsimd.tensor_tensor(sz, e2, zx, AOP.mult)
            st.silu_z[zi] = sz
        if mi % 2 == 1:
            yield

    # ---- conv weights / bias (single DMAs)
    wc = pp.wpool.tile([128, 4 * NDI], FP32, name=f"wc_{pfx}", tag=f"wc_{pfx}", bufs=1)
    nc.sync.dma_start(wc.rearrange("p (o k) -> p o k", o=NDI),
                        d("conv_w").rearrange("(o p) k -> p o k", p=128))
    cb = pp.wpool.tile([128, NDI], FP32, name=f"cb_{pfx}", tag=f"cb_{pfx}", bufs=1)
    nc.sync.dma_start(cb, _dcols(d("conv_b")))
    cbn = None
    if not use_silu_table:
        cbn = pp.wpool.tile([128, NDI], FP32, name=f"cbn_{pfx}", tag=f"cbn_{pfx}", bufs=1)
        nc.vector.tensor_scalar_mul(cbn, cb, -1.0)
    # dt_b / D / A columns
    st.db = pp.wpool.tile([128, NDI], FP32, name=f"db_{pfx}", tag=f"db_{pfx}", bufs=1)
    nc.sync.dma_start(st.db, _dcols(d("dt_b")))
    st.Dc = pp.wpool.tile([128, NDI], FP32, name=f"Dc_{pfx}", tag=f"Dc_{pfx}", bufs=1)
    nc.sync.dma_start(st.Dc, _dcols(d("D")))
    al = pp.wpool.tile([128, NDI * DS], FP32, name=f"al_{pfx}", tag=f"al_{pfx}", bufs=1)
    nc.sync.dma_start(al.rearrange("p (o n) -> p o n", o=NDI),
                        d("A_log").rearrange("(o p) n -> p o n", p=128))
    yield

    # ---- causal depthwise conv (PE diag matmuls) + silu -> u
    st.u = [None] * NDI
    for di in range(NDI):
        ps = pp.pwork.tile([128, L], FP32, name=f"ps_cv_{pfx}_{di}", tag="work")
        for j in range(4):
            dg = pp.wpool.tile([128, 128], BF16, name=f"dg_{pfx}_{di}_{j}", tag="dg", bufs=5)
            jj = j if not rev else 3 - j
            nc.vector.tensor_scalar_mul(dg, pp.ident, wc[:, di * 4 + jj:di * 4 + jj + 1])
            if not rev:
                s = 3 - jj
                rhs = st.u_pad[di][:, 3 - s:3 - s + L]
            else:
                rhs = st.u_pad[di][:, 3 + j:3 + j + L]
            nc.tensor.matmul(ps, dg, rhs, start=(j == 0), stop=(j == 3))
        ut = pp.bpool.tile([128, L], BF16, name=f"u_{pfx}_{di}", tag=f"u_{pfx}{di}", bufs=1)
        if use_silu_table:
            nc.scalar.activation(ut, ps, AF.Silu, bias=cb[:, di:di + 1])
        else:
            ux = pp.spool.tile([128, L], BF16, name=f"ux_{pfx}_{di}", tag="fe_t1", bufs=1)
            nc.scalar.activation(ux, ps, AF.Identity, bias=cb[:, di:di + 1])
            e1 = pp.spool.tile([128, L], BF16, name=f"e1u_{pfx}_{di}", tag="fe_t2", bufs=1)
            nc.scalar.activation(e1, ps, AF.Exp, scale=-1.0, bias=cbn[:, di:di + 1])
            sp = pp.spool.tile([128, L], BF16, name=f"spu_{pfx}_{di}", tag="fe_t3", bufs=1)
            nc.scalar.activation(sp, e1, AF.Ln, bias=pp.ones_col)
            e2 = pp.spool.tile([128, L], BF16, name=f"e2u_{pfx}_{di}", tag="fe_t2", bufs=1)
            nc.scalar.activation(e2, sp, AF.Exp, scale=-1.0)
            nc.gpsimd.tensor_tensor(ut, e2, ux, AOP.mult)
        st.u[di] = ut
        if di % 2 == 1:
            yield

    # ---- xproj: dbc^T [64, 512] = xproj_w tiles @ u
    xproj_wT = ins[pfx + "_xproj_wT"]
    ps_dbc = pp.pwork.tile([64, L], FP32, name=f"ps_dbc_{pfx}", tag="work")
    for ki in range(NDI):
        wb = pp.wpool.tile([128, 64], BF16, name=f"w_xp_{pfx}_{ki}", tag="wxb", bufs=4)
        nc.sync.dma_start(wb, xproj_wT[ts(ki, 128), :])
        nc.tensor.matmul(ps_dbc, wb, st.u[ki], start=(ki == 0), stop=(ki == NDI - 1))
    dbc = pp.bpool.tile([64, L], BF16, name=f"dbc_{pfx}", tag="dbc", bufs=1)
    nc.scalar.activation(dbc, ps_dbc, AF.Copy)
    yield

    st.dbc = dbc

    # ---- A = -exp(A_log) [128, 8*16]
    ae = pp.wpool.tile([128, NDI * DS], FP32, name=f"ae_{pfx}", tag=f"ae_{pfx}", bufs=1)
    nc.scalar.activation(ae, al, AF.Exp)
    st.A = pp.cpool.tile([128, NDI * DS], FP32, name=f"A_{pfx}", tag=f"A_{pfx}")
    nc.vector.tensor_scalar_mul(st.A, ae, -1.0)
    yield

    # ---- dt_proj + softplus -> delta ; du = delta*u (gpsimd)
    dt_wT = ins[pfx + "_dt_wT"]
    st.delta = [None] * NDI
    st.du = [None] * NDI
    for di in range(NDI):
        wb = pp.wpool.tile([32, 128], BF16, name=f"w_dt_{pfx}_{di}", tag="wdb", bufs=4)
        nc.sync.dma_start(wb, dt_wT[:, ts(di, 128)])
        ps = pp.pwork.tile([128, L], FP32, name=f"ps_dt_{pfx}_{di}", tag="work")
        nc.tensor.matmul(ps, wb, dbc[0:DTR, :], start=True, stop=True)
        ed = pp.spool.tile([128, L], BF16, name=f"ed_{pfx}_{di}", tag="fe_t1", bufs=1)
        nc.scalar.activation(ed, ps, AF.Exp, bias=st.db[:, di:di + 1])
        dl = pp.bpool.tile([128, L], BF16, name=f"delta_{pfx}_{di}",
                           tag=f"delta_{pfx}{di}", bufs=1)
        nc.scalar.activation(dl, ed, AF.Ln, bias=pp.ones_col)
        st.delta[di] = dl
        dut = pp.bpool.tile([128, L], BF16, name=f"du_{pfx}_{di}", tag=f"du_{pfx}{di}", bufs=1)
        nc.gpsimd.tensor_tensor(dut, dl, st.u[di], AOP.mult)
        st.du[di] = dut
        if di % 2 == 1:
            yield


def _scan_units(tc, pp, pfx, rev, op_accum):
    """Generator: fwd/rev scan block, one quad per yield.

    op_accum: if not None, (out_wT, ps_op list) - accumulate out_proj over
    d-tiles in PSUM as each yg completes (ki-outer).
    """
    nc = tc.nc
    st = pp.st[pfx]

    # ---- broadcasts: Brep/Crep [128, 16*512] via JIT flat rows on partition 0
    Brep = pp.bpool.tile([128, DS * L], BF16, name=f"Brep_{pfx}", tag="Brep", bufs=1)
    Crep = pp.bpool.tile([128, DS * L], BF16, name=f"Crep_{pfx}", tag="Crep", bufs=1)
    for q in range(DS // NQ):
        fb = pp.spool.tile([1, QW], BF16, name=f"flatB_{pfx}_{q}", tag="flatq", bufs=1)
        for j in range(NQ):
            n = q * NQ + j
            nc.sync.dma_start(fb[:, ts(j, L)], st.dbc[DTR + n:DTR + n + 1, :])
        nc.gpsimd.partition_broadcast(Brep[:, ts(q, QW)], fb)
        fc = pp.spool.tile([1, QW], BF16, name=f"flatC_{pfx}_{q}", tag="flatq", bufs=1)
        for j in range(NQ):
            n = q * NQ + j
            nc.sync.dma_start(fc[:, ts(j, L)], st.dbc[DTR + DS + n:DTR + DS + n + 1, :])
        nc.gpsimd.partition_broadcast(Crep[:, ts(q, QW)], fc)
    yield

    st.yg = [None] * NDI
    for di in range(NDI):
        ps_y = pp.pyacc.tile([128, L], FP32, name=f"ps_y_{pfx}_{di}", tag="psy")
        du_rep = st.du[di].unsqueeze(1).broadcast_to((128, NQ, L))
        for q in range(DS // NQ):
            # dA quad: 4 activations (skip the scan-boundary column), 4 memsets
            dAq = pp.spool.tile([128, QW], BF16, name=f"dA_{pfx}_{di}_{q}", tag="dAq")
            for j in range(NQ):
                n = q * NQ + j
                sc = st.A[:, di * DS + n:di * DS + n + 1]
                if not rev:
                    nc.vector.memset(dAq[:, j * L:j * L + 1], 0)
                    nc.scalar.activation(dAq[:, j * L + 1:(j + 1) * L],
                                         st.delta[di][:, 1:L], AF.Exp, scale=sc)
                else:
                    nc.vector.memset(dAq[:, (j + 1) * L - 1:(j + 1) * L], 0)
                    nc.scalar.activation(dAq[:, j * L:(j + 1) * L - 1],
                                         st.delta[di][:, 0:L - 1], AF.Exp, scale=sc)
            # dBu quad on gpsimd
            dBuq = pp.spool.tile([128, QW], BF16, name=f"dBu_{pfx}_{di}_{q}", tag="dBuq")
            nc.gpsimd.tensor_tensor(dBuq.rearrange("p (q f) -> p q f", q=NQ),
                                    du_rep,
                                    Brep[:, ts(q, QW)].rearrange("p (q f) -> p q f", q=NQ),
                                    AOP.mult)
            # merged segmented scan
            hq = pp.spool.tile([128, QW], BF16, name=f"h_{pfx}_{di}_{q}", tag="hq")
            if not rev:
                nc.vector.tensor_tensor_scan(hq, dAq, dBuq, 0.0, AOP.mult, AOP.add)
            else:
                nc.vector.tensor_tensor_scan(hq[:, ::-1], dAq[:, ::-1], dBuq[:, ::-1],
                                             0.0, AOP.mult, AOP.add)
            # hC mult (in place) + y-reduce matmuls
            nc.vector.tensor_mul(hq, hq, Crep[:, ts(q, QW)])
            for j in range(NQ):
                n = q * NQ + j
                nc.tensor.matmul(ps_y, pp.ident, hq[:, ts(j, L)],
                                 start=(n == 0), stop=(n == DS - 1))
            yield

        # y += u*D ; gate with silu(z)
        yD = pp.spool.tile([128, L], BF16, name=f"yD_{pfx}_{di}", tag="yD", bufs=1)
        nc.vector.scalar_tensor_tensor(yD, st.u[di], st.Dc[:, di:di + 1], ps_y,
                                       AOP.mult, AOP.add)
        ygt = pp.bpool.tile([128, L], BF16, name=f"yg_{pfx}_{di}", tag=f"yg{di}", bufs=1)
        nc.vector.tensor_mul(ygt, yD, st.silu_z[di])
        st.yg[di] = ygt
        if op_accum is not None:
            out_wT, ps_op = op_accum
            for mi in range(NDM):
                w = pp.wpool.tile([128, 128], BF16, name=f"w_op_{pfx}_{di}_{mi}", tag="wo", bufs=8)
                nc.sync.dma_start(w, out_wT[ts(di, 128), ts(mi, 128)])
                nc.tensor.matmul(ps_op[mi], w, ygt, start=(di == 0), stop=(di == NDI - 1))


def _out_proj_mi(tc, pp, pfx, ins, xTb):
    """out_proj with mi-outer loop; x-residual folded in as an identity MM."""
    nc = tc.nc
    st = pp.st[pfx]
    out_wT = ins[pfx + "_out_wT"]
    new_cur = []
    for mi in range(NDM):
        ps = pp.pwork.tile([128, L], FP32, name=f"ps_op_{pfx}_{mi}", tag="work")
        for ki in range(NDI):
            w = pp.wpool.tile([128, 128], BF16, name=f"w_op_{pfx}_{mi}_{ki}", tag="wo", bufs=8)
            nc.sync.dma_start(w, out_wT[ts(ki, 128), ts(mi, 128)])
            nc.tensor.matmul(ps, w, st.yg[ki], start=(ki == 0), stop=False)
        nc.tensor.matmul(ps, pp.ident, xTb[mi], start=False, stop=True)
        ncur = pp.bpool.tile([128, L], FP32, name=f"cur_{pfx}_{mi}", tag=f"cur{mi}", bufs=1)
        nc.scalar.activation(ncur, ps, AF.Copy)
        new_cur.append(ncur)
    return new_cur


def _layernorm(tc, pp, x_tiles, g_ap, b_ap, name, keep_all=True):
    """LN over the partition(feature) axis of transposed tiles, via PE colsums."""
    nc = tc.nc
    bpool, pwork = pp.bpool, pp.pwork
    ones = pp.cpool.tile([128, 1], BF16, name=f"ones_{name}", tag="ones")
    nc.vector.memset(ones, 1.0)
    ps_s = pwork.tile([1, L], FP32, name=f"ps_s_{name}", tag="work")
    ps_q = pwork.tile([1, L], FP32, name=f"ps_q_{name}", tag="work")
    for ki in range(NDM):
        xb = bpool.tile([128, L], BF16, name=f"xb_{name}_{ki}", tag="lnxb")
        nc.scalar.activation(xb, x_tiles[ki], AF.Copy)
        nc.tensor.matmul(ps_s, ones, xb, start=(ki == 0), stop=(ki == NDM - 1))
        sq = bpool.tile([128, L], BF16, name=f"sq_{name}_{ki}", tag="sq")
        nc.scalar.activation(sq, x_tiles[ki], AF.Square)
        nc.tensor.matmul(ps_q, ones, sq, start=(ki == 0), stop=(ki == NDM - 1))
    mean = bpool.tile([1, L], FP32, name=f"mean_{name}", tag="st1")
    nc.vector.tensor_scalar_mul(mean, ps_s, 1.0 / DM)
    msq = bpool.tile([1, L], FP32, name=f"msq_{name}", tag="st2")
    nc.vector.tensor_mul(msq, mean, mean)
    var = bpool.tile([1, L], FP32, name=f"var_{name}", tag="st3")
    nc.vector.scalar_tensor_tensor(var, ps_q, 1.0 / DM, msq, AOP.mult, AOP.subtract)
    sd = bpool.tile([1, L], FP32, name=f"sd_{name}", tag="st4")
    epsc = bpool.tile([1, 1], FP32, name=f"eps_{name}", tag="eps")
    nc.vector.memset(epsc, 1e-5)
    nc.scalar.activation(sd, var, AF.Sqrt, bias=epsc)
    istd = bpool.tile([1, L], FP32, name=f"istd_{name}", tag="st5")
    nc.vector.reciprocal(istd, sd)
    shift = bpool.tile([1, L], FP32, name=f"shift_{name}", tag="st6")
    nc.vector.tensor_mul(shift, mean, istd)
    nc.vector.tensor_scalar_mul(shift, shift, -1.0)
    istd_r = bpool.tile([128, L], FP32, name=f"istd_r_{name}", tag="istd_r", bufs=1)
    shift_r = bpool.tile([128, L], FP32, name=f"shift_r_{name}", tag="shift_r", bufs=1)
    nc.gpsimd.partition_broadcast(istd_r, istd)
    nc.gpsimd.partition_broadcast(shift_r, shift)
    gc = bpool.tile([128, NDM], FP32, name=f"g_{name}", tag=f"g_{name}")
    nc.sync.dma_start(gc, _mcols(g_ap, NDM))
    bc = bpool.tile([128, NDM], FP32, name=f"b_{name}", tag=f"b_{name}")
    nc.sync.dma_start(bc, _mcols(b_ap, NDM))
    out_tiles = []
    for ki in range(NDM):
        t1 = bpool.tile([128, L], FP32, name=f"t1_{name}_{ki}", tag="lnt1")
        nc.vector.tensor_mul(t1, x_tiles[ki], istd_r)
        nc.vector.tensor_add(t1, t1, shift_r)
        if keep_all:
            t3 = bpool.tile([128, L], BF16, name=f"t3_{name}_{ki}", tag=f"xTb{ki}")
        else:
            t3 = bpool.tile([128, L], FP32, name=f"t3_{name}_{ki}", tag="ln_out", bufs=2)
        nc.scalar.activation(t3, t1, AF.Identity, scale=gc[:, ki:ki + 1],
                             bias=bc[:, ki:ki + 1])
        out_tiles.append(t3)
    return out_tiles


def _drain(gen):
    for _ in gen:
        pass


def _kernel(tc, out_d, ins):
    nc = tc.nc
    pp = P()
    with (tc.tile_pool(name="const", bufs=1) as cpool,
          tc.tile_pool(name="big", bufs=1) as bpool,
          tc.tile_pool(name="wts", bufs=2) as wpool,
          tc.tile_pool(name="scan", bufs=2) as spool,
          tc.tile_pool(name="pwork", bufs=2, space="PSUM") as pwork,
          tc.tile_pool(name="pyacc", bufs=2, space="PSUM") as pyacc,
          tc.tile_pool(name="pacc4", bufs=4, space="PSUM") as pacc4):
        pp.cpool, pp.bpool, pp.wpool, pp.spool = cpool, bpool, wpool, spool
        pp.pwork, pp.pyacc, pp.pacc4 = pwork, pyacc, pacc4
        pp.st = {}

        pp.ident = cpool.tile([128, 128], BF16, name="ident", tag="ident")
        make_identity(nc, pp.ident)
        pp.ones_col = cpool.tile([128, 1], FP32, name="ones_col", tag="ones_col")
        nc.vector.memset(pp.ones_col, 1.0)

        xTb = []
        for i in range(NDM):
            xtb = bpool.tile([128, L], BF16, name=f"xTb_{i}", tag=f"xTb{i}")
            nc.scalar.dma_start(xtb, ins["xbT16"][ts(i, 128), :])
            xTb.append(xtb)

        # ---- phase A: fwd front-end (uses the Silu table, ends in nat_log_exp)
        _drain(_frontend_units(tc, pp, ins, xTb, "f", False, use_silu_table=True))

        # ---- phase B: fwd scan block interleaved with rev front-end
        fe_r = _frontend_units(tc, pp, ins, xTb, "r", True, use_silu_table=False)
        for _ in _scan_units(tc, pp, "f", False, op_accum=None):
            next(fe_r, None)
        _drain(fe_r)

        # ---- phase C: fwd out_proj (mi-outer) + rev scan block with fused
        #      rev out_proj accumulation (ki-outer, pacc4)
        cur1 = _out_proj_mi(tc, pp, "f", ins, xTb)
        ps_opr = [pacc4.tile([128, L], FP32, name=f"ps_opr_{mi}", tag="acc4")
                  for mi in range(NDM)]
        _drain(_scan_units(tc, pp, "r", True, op_accum=(ins["r_out_wT"], ps_opr)))
        cur2 = cur1
        for mi in range(NDM):
            nc.vector.scalar_tensor_tensor(cur2[mi], ps_opr[mi], 1.0, cur1[mi],
                                           AOP.mult, AOP.add)

        # ---- phase D: LN1, FFN, LN2
        x1 = _layernorm(tc, pp, cur2, ins["ln1_g"], ins["ln1_b"], "ln1")

        conv1_wT = ins["conv1_wT"]
        conv2_wT = ins["conv2_wT"]
        c1b = wpool.tile([128, NFF], FP32, name="c1b", tag="c1b", bufs=1)
        nc.sync.dma_start(c1b, _mcols(ins["conv1_b"], NFF))
        c2b = wpool.tile([128, NDM], FP32, name="c2b", tag="c2b", bufs=1)
        nc.sync.dma_start(c2b, _mcols(ins["conv2_b"], NDM))
        x1b = x1
        ps2 = [pacc4.tile([128, L], FP32, name=f"ps_ffn_{mi}", tag="acc4")
               for mi in range(NDM)]
        for ffi in range(NFF):
            ps1 = pwork.tile([128, L], FP32, name=f"ps_ff1_{ffi}", tag="work")
            for ki in range(NDM):
                w = wpool.tile([128, 128], BF16, name=f"w_f1_{ffi}_{ki}", tag="wf1", bufs=12)
                nc.sync.dma_start(w, conv1_wT[ts(ki, 128), ts(ffi, 128)])
                nc.tensor.matmul(ps1, w, x1b[ki], start=(ki == 0), stop=(ki == NDM - 1))
            y1 = bpool.tile([128, L], BF16, name=f"y1_{ffi}", tag="y1", bufs=2)
            nc.scalar.activation(y1, ps1, AF.Gelu, bias=c1b[:, ffi:ffi + 1])
            for mi in range(NDM):
                w2 = wpool.tile([128, 128], BF16, name=f"w_f2_{ffi}_{mi}", tag="wf2", bufs=12)
                nc.sync.dma_start(w2, conv2_wT[ts(ffi, 128), ts(mi, 128)])
                nc.tensor.matmul(ps2[mi], w2, y1, start=(ffi == 0), stop=(ffi == NFF - 1))
        x2 = []
        for mi in range(NDM):
            t = bpool.tile([128, L], FP32, name=f"x2_{mi}", tag=f"x2_{mi}")
            nc.vector.scalar_tensor_tensor(t, ps2[mi], 1.0, x1[mi], AOP.mult, AOP.add)
            nc.scalar.activation(t, t, AF.Identity, bias=c2b[:, mi:mi + 1])
            x2.append(t)

        out_t = _layernorm(tc, pp, x2, ins["ln2_g"], ins["ln2_b"], "ln2", keep_all=False)
        for mi in range(NDM):
            nc.scalar.dma_start(out_d[ts(mi, 128), :], out_t[mi])


@functools.lru_cache(maxsize=1)
def _build():
    nc = bacc.Bacc("TRN2", debug=False)
    ins = {"xbT": nc.dram_tensor("xbT", (DM, L), FP32, kind="ExternalInput").ap()}
    for name, shape in W_SHAPES.items():
        ins[name] = nc.dram_tensor(name, shape, FP32, kind="ExternalInput").ap()
    for name, shape in T_SHAPES.items():
        ins[name] = nc.dram_tensor(name, shape, BF16, kind="ExternalInput").ap()
    ins["xbT16"] = nc.dram_tensor("xbT16", (DM, L), BF16, kind="ExternalInput").ap()
    out_d = nc.dram_tensor("out", (DM, L), FP32, kind="ExternalOutput").ap()
    with tile.TileContext(nc) as tc:
        _kernel(tc, out_d, ins)
    nc.compile()
    return nc


def make_in_maps(inputs):
    import ml_dtypes
    bf = ml_dtypes.bfloat16
    shared = {}
    for name in W_SHAPES:
        shared[name] = np.ascontiguousarray(inputs[name], dtype=np.float32)
    for tname, sname in T_SOURCES.items():
        shared[tname] = np.ascontiguousarray(
            np.asarray(inputs[sname], dtype=np.float32).T).astype(bf)
    in_maps = []
    for c in range(NB):
        m = dict(shared)
        xt = np.ascontiguousarray(np.asarray(inputs["x"][c], dtype=np.float32).T)
        m["xbT"] = xt
        m["xbT16"] = xt.astype(bf)
        in_maps.append(m)
    return in_maps


def kernel(**inputs):
    nc = _build()
    res = run_bass_kernel_spmd(nc, make_in_maps(inputs), list(range(NB)))
    return np.stack([res.results[c]["out"].T for c in range(NB)]).astype(np.float32)


# revision 10
# speedup vs baseline: 1.5791x; 1.5791x over previous
"""Trainium2 Bass kernel for a bidirectional-Mamba decoder layer.

Sharding: data-parallel over batch, one sequence per NeuronCore (B=8, 8 cores).
Layout: transposed throughout (features on partitions, time on free dim).

Schedule (per core):
  A: fwd front-end (in_proj/conv/xproj/dt)           - tensor/scalar
  B: fwd scan block (vector+gpsimd+scalar) overlapped with rev front-end
  C: rev scan block overlapped with fwd+rev out_proj  - vector-bound
  D: LN1 + FFN + LN2 tail                             - tensor/scalar

Engine split in scan blocks: vector = merged 2048-wide segmented scans + hC
mult; gpsimd = du mult, dBu mult (broadcast-AP), B/C partition-broadcasts;
scalar = dA exponentials; tensor = y-reduce identity matmuls + out_proj.
"""
import sys
sys.path.insert(0, "/opt/trn_rl_repo")

import functools
import numpy as np

import concourse.bass as bass
import concourse.mybir as mybir
import concourse.tile as tile
from concourse import bacc
from concourse.bass import ts
from concourse.bass_utils import run_bass_kernel_spmd
from concourse.masks import make_identity

# Restrict activation-table choice so the table-load pass doesn't ping-pong
# between equivalent tables. Index order must be preserved (the emitted
# act_func_set_id is the index into act_info.json), so unwanted tables are
# emptied in place rather than removed.
import concourse.hw_specs as _hw_specs
_KEEP_TABLES = {"natural_log_exp_and_others", "sqrt_and_others",
                "gelu_and_others", "silu_and_others"}
_orig_get_tables = _hw_specs.get_activation_tables
_tab_cache = {}


def _filtered_tables(arch):
    if arch not in _tab_cache:
        t = _orig_get_tables(arch)
        _tab_cache[arch] = {k: (v if k in _KEEP_TABLES else set()) for k, v in t.items()}
    return _tab_cache[arch]


_hw_specs.get_activation_tables = _filtered_tables
import concourse.bacc as _bacc_mod
_bacc_mod.get_activation_tables = _filtered_tables

FP32 = mybir.dt.float32
BF16 = mybir.dt.bfloat16
AOP = mybir.AluOpType
AF = mybir.ActivationFunctionType

DM, DI, DS, DTR, DFF, L = 512, 1024, 16, 32, 2048, 512
NDM, NDI, NFF = DM // 128, DI // 128, DFF // 128   # 4, 8, 16
NB = 8   # batch == cores
NQ = 4   # n-states per merged scan quad
QW = NQ * L  # 2048

W_SHAPES = {}
for p in ("f", "r"):
    W_SHAPES.update({
        p + "_in_w": (2 * DI, DM), p + "_conv_w": (DI, 4), p + "_conv_b": (DI,),
        p + "_xproj_w": (DTR + 2 * DS, DI), p + "_dt_w": (DI, DTR), p + "_dt_b": (DI,),
        p + "_A_log": (DI, DS), p + "_D": (DI,), p + "_out_w": (DM, DI),
    })
W_SHAPES.update({
    "conv1_w": (DFF, DM), "conv1_b": (DFF,), "conv2_w": (DM, DFF), "conv2_b": (DM,),
    "ln1_g": (DM,), "ln1_b": (DM,), "ln2_g": (DM,), "ln2_b": (DM,),
})
T_SHAPES = {}
for p in ("f", "r"):
    T_SHAPES.update({
        p + "_in_wT": (DM, 2 * DI), p + "_xproj_wT": (DI, DTR + 2 * DS),
        p + "_dt_wT": (DTR, DI), p + "_out_wT": (DI, DM),
    })
T_SHAPES.update({"conv1_wT": (DM, DFF), "conv2_wT": (DFF, DM)})
T_SOURCES = {n: n[:-1] for n in T_SHAPES}  # strip trailing T -> source name


def _dcols(ap_1d):
    """(1024,) DRAM tensor -> [128, 8] AP (d-tile index on free dim)."""
    return ap_1d.rearrange("(o p) -> p o", p=128)


def _mcols(ap_1d, n):
    """(n*128,) DRAM tensor -> [128, n] AP."""
    return ap_1d.rearrange("(o p) -> p o", p=128)


class P:
    """Pool/shared-state holder."""
    pass


def _frontend_units(tc, pp, ins, xTb, pfx, rev, use_silu_table):
    """Generator: emits one front-end unit per yield. Fills pp.st[pfx]."""
    nc = tc.nc
    st = pp.st[pfx] = P()
    d = lambda name: ins[pfx + "_" + name]

    # ---- in_proj: xz^T [2048, 512] = in_w^T-tiles @ x^T (batched weight DMAs)
    in_wT = ins[pfx + "_in_wT"]
    st.u_pad = [None] * NDI
    st.silu_z = [None] * NDI
    chunks = {}
    for mi in range(2 * NDI):
        half = mi // 8
        if (half, 0) not in chunks:
            for ki in range(NDM):
                ch = pp.wpool.tile([128, 1024], BF16, name=f"inw_{pfx}_{half}_{ki}",
                                   tag="inw", bufs=5)
                nc.sync.dma_start(ch, in_wT[ts(ki, 128), half * 1024:(half + 1) * 1024])
                chunks[(half, ki)] = ch
        ps = pp.pwork.tile([128, L], FP32, name=f"ps_in_{pfx}_{mi}", tag="work")
        for ki in range(NDM):
            w = chunks[(half, ki)][:, (mi - half * 8) * 128:(mi - half * 8 + 1) * 128]
            nc.tensor.matmul(ps, w, xTb[ki], start=(ki == 0), stop=(ki == NDM - 1))
        if mi < NDI:
            up = pp.bpool.tile([128, L + 6], BF16, name=f"u_pad_{pfx}_{mi}",
                               tag=f"u_pad{mi}", bufs=1)
            nc.vector.memset(up[:, 0:3], 0)
            nc.vector.memset(up[:, L + 3:L + 6], 0)
            nc.scalar.activation(up[:, 3:L + 3], ps, AF.Copy)
            st.u_pad[mi] = up
        else:
            zi = mi - NDI
            sz = pp.bpool.tile([128, L], BF16, name=f"silu_z_{pfx}_{zi}",
                               tag=f"silu_z_{pfx}{zi}", bufs=1)
            if use_silu_table:
                nc.scalar.activation(sz, ps, AF.Silu)
            else:
                # exp/ln chain keeps the scalar engine in natural_log_exp
                zx = pp.spool.tile([128, L], BF16, name=f"zx_{pfx}_{zi}", tag="fe_t1", bufs=1)
                nc.scalar.activation(zx, ps, AF.Copy)
                e1 = pp.spool.tile([128, L], BF16, name=f"e1z_{pfx}_{zi}", tag="fe_t2", bufs=1)
                nc.scalar.activation(e1, ps, AF.Exp, scale=-1.0)
                sp = pp.spool.tile([128, L], BF16, name=f"spz_{pfx}_{zi}", tag="fe_t3", bufs=1)
                nc.scalar.activation(sp, e1, AF.Ln, bias=pp.ones_col)
                e2 = pp.spool.tile([128, L], BF16, name=f"e2z_{pfx}_{zi}", tag="fe_t2", bufs=1)
                nc.scalar.activation(e2, sp, AF.Exp, scale=-1.0)
                nc.vector.tensor_mul(sz, e2, zx)
            st.silu_z[zi] = sz
        if mi % 2 == 1:
            yield

    # ---- conv weights / bias (single DMAs)
    wc = pp.wpool.tile([128, 4 * NDI], FP32, name=f"wc_{pfx}", tag=f"wc_{pfx}", bufs=1)
    nc.sync.dma_start(wc.rearrange("p (o k) -> p o k", o=NDI),
                        d("conv_w").rearrange("(o p) k -> p o k", p=128))
    cb = pp.wpool.tile([128, NDI], FP32, name=f"cb_{pfx}", tag=f"cb_{pfx}", bufs=1)
    nc.sync.dma_start(cb, _dcols(d("conv_b")))
    cbn = None
    if not use_silu_table:
        cbn = pp.wpool.tile([128, NDI], FP32, name=f"cbn_{pfx}", tag=f"cbn_{pfx}", bufs=1)
        nc.vector.tensor_scalar_mul(cbn, cb, -1.0)
    # dt_b / D / A columns
    st.db = pp.wpool.tile([128, NDI], FP32, name=f"db_{pfx}", tag=f"db_{pfx}", bufs=1)
    nc.sync.dma_start(st.db, _dcols(d("dt_b")))
    st.Dc = pp.wpool.tile([128, NDI], FP32, name=f"Dc_{pfx}", tag=f"Dc_{pfx}", bufs=1)
    nc.sync.dma_start(st.Dc, _dcols(d("D")))
    al = pp.wpool.tile([128, NDI * DS], FP32, name=f"al_{pfx}", tag=f"al_{pfx}", bufs=1)
    nc.sync.dma_start(al.rearrange("p (o n) -> p o n", o=NDI),
                        d("A_log").rearrange("(o p) n -> p o n", p=128))
    yield

    # ---- causal depthwise conv (PE diag matmuls) + silu -> u
    st.u = [None] * NDI
    for di in range(NDI):
        ps = pp.pwork.tile([128, L], FP32, name=f"ps_cv_{pfx}_{di}", tag="work")
        for j in range(4):
            dg = pp.wpool.tile([128, 128], BF16, name=f"dg_{pfx}_{di}_{j}", tag="dg", bufs=5)
            jj = j if not rev else 3 - j
            nc.vector.tensor_scalar_mul(dg, pp.ident, wc[:, di * 4 + jj:di * 4 + jj + 1])
            if not rev:
                s = 3 - jj
                rhs = st.u_pad[di][:, 3 - s:3 - s + L]
            else:
                rhs = st.u_pad[di][:, 3 + j:3 + j + L]
            nc.tensor.matmul(ps, dg, rhs, start=(j == 0), stop=(j == 3))
        ut = pp.bpool.tile([128, L], BF16, name=f"u_{pfx}_{di}", tag=f"u_{pfx}{di}", bufs=1)
        if use_silu_table:
            nc.scalar.activation(ut, ps, AF.Silu, bias=cb[:, di:di + 1])
        else:
            ux = pp.spool.tile([128, L], BF16, name=f"ux_{pfx}_{di}", tag="fe_t1", bufs=1)
            nc.scalar.activation(ux, ps, AF.Identity, bias=cb[:, di:di + 1])
            e1 = pp.spool.tile([128, L], BF16, name=f"e1u_{pfx}_{di}", tag="fe_t2", bufs=1)
            nc.scalar.activation(e1, ps, AF.Exp, scale=-1.0, bias=cbn[:, di:di + 1])
            sp = pp.spool.tile([128, L], BF16, name=f"spu_{pfx}_{di}", tag="fe_t3", bufs=1)
            nc.scalar.activation(sp, e1, AF.Ln, bias=pp.ones_col)
            e2 = pp.spool.tile([128, L], BF16, name=f"e2u_{pfx}_{di}", tag="fe_t2", bufs=1)
            nc.scalar.activation(e2, sp, AF.Exp, scale=-1.0)
            nc.vector.tensor_mul(ut, e2, ux)
        st.u[di] = ut
        if di % 2 == 1:
            yield

    # ---- xproj: dbc^T [64, 512] = xproj_w tiles @ u
    xproj_wT = ins[pfx + "_xproj_wT"]
    ps_dbc = pp.pwork.tile([64, L], FP32, name=f"ps_dbc_{pfx}", tag="work")
    for ki in range(NDI):
        wb = pp.wpool.tile([128, 64], BF16, name=f"w_xp_{pfx}_{ki}", tag="wxb", bufs=4)
        nc.sync.dma_start(wb, xproj_wT[ts(ki, 128), :])
        nc.tensor.matmul(ps_dbc, wb, st.u[ki], start=(ki == 0), stop=(ki == NDI - 1))
    dbc = pp.bpool.tile([64, L], BF16, name=f"dbc_{pfx}", tag="dbc", bufs=1)
    nc.scalar.activation(dbc, ps_dbc, AF.Copy)
    yield

    st.dbc = dbc

    # ---- A = -exp(A_log) [128, 8*16]
    ae = pp.wpool.tile([128, NDI * DS], FP32, name=f"ae_{pfx}", tag=f"ae_{pfx}", bufs=1)
    nc.scalar.activation(ae, al, AF.Exp)
    st.A = pp.cpool.tile([128, NDI * DS], FP32, name=f"A_{pfx}", tag=f"A_{pfx}")
    nc.vector.tensor_scalar_mul(st.A, ae, -1.0)
    yield

    # ---- dt_proj + softplus -> delta ; du = delta*u (gpsimd)
    dt_wT = ins[pfx + "_dt_wT"]
    st.delta = [None] * NDI
    st.du = [None] * NDI
    for di in range(NDI):
        wb = pp.wpool.tile([32, 128], BF16, name=f"w_dt_{pfx}_{di}", tag="wdb", bufs=4)
        nc.sync.dma_start(wb, dt_wT[:, ts(di, 128)])
        ps = pp.pwork.tile([128, L], FP32, name=f"ps_dt_{pfx}_{di}", tag="work")
        nc.tensor.matmul(ps, wb, dbc[0:DTR, :], start=True, stop=True)
        ed = pp.spool.tile([128, L], BF16, name=f"ed_{pfx}_{di}", tag="fe_t1", bufs=1)
        nc.scalar.activation(ed, ps, AF.Exp, bias=st.db[:, di:di + 1])
        dl = pp.bpool.tile([128, L], BF16, name=f"delta_{pfx}_{di}",
                           tag=f"delta_{pfx}{di}", bufs=1)
        nc.scalar.activation(dl, ed, AF.Ln, bias=pp.ones_col)
        st.delta[di] = dl
        dut = pp.bpool.tile([128, L], BF16, name=f"du_{pfx}_{di}", tag=f"du_{pfx}{di}", bufs=1)
        nc.vector.tensor_mul(dut, dl, st.u[di])
        st.du[di] = dut
        if di % 2 == 1:
            yield


def _scan_units(tc, pp, pfx, rev, op_accum):
    """Generator: fwd/rev scan block, one quad per yield.

    op_accum: if not None, (out_wT, ps_op list) - accumulate out_proj over
    d-tiles in PSUM as each yg completes (ki-outer).
    """
    nc = tc.nc
    st = pp.st[pfx]

    # ---- broadcasts: Brep/Crep [128, 16*512] via PE ones-matmul + scalar copy
    Brep = pp.bpool.tile([128, DS * L], BF16, name=f"Brep_{pfx}", tag="Brep", bufs=1)
    Crep = pp.bpool.tile([128, DS * L], BF16, name=f"Crep_{pfx}", tag="Crep", bufs=1)
    for hq2 in range(DS // 2):
        rep, base = (Brep, DTR) if hq2 < DS // 4 else (Crep, DTR + DS)
        nb = (hq2 % (DS // 4)) * 2
        fb = pp.spool.tile([1, 2 * L], BF16, name=f"flat_{pfx}_{hq2}", tag="flatq", bufs=1)
        for j in range(2):
            nc.sync.dma_start(fb[:, ts(j, L)], st.dbc[base + nb + j:base + nb + j + 1, :])
        for j in range(2):
            ps = pp.pwork.tile([128, L], FP32, name=f"psbc_{pfx}_{hq2}_{j}", tag="work")
            nc.tensor.matmul(ps, pp.ones_w, fb[:, ts(j, L)], start=True, stop=True)
            nc.scalar.activation(rep[:, ts(nb + j, L)], ps, AF.Copy)
    yield

    st.yg = [None] * NDI
    for di in range(NDI):
        ps_y = pp.pyacc.tile([128, L], FP32, name=f"ps_y_{pfx}_{di}", tag="psy")
        du_rep = st.du[di].unsqueeze(1).broadcast_to((128, NQ, L))
        for q in range(DS // NQ):
            # dA quad: 4 activations (skip the scan-boundary column), 1 strided memset
            dAq = pp.spool.tile([128, QW], BF16, name=f"dA_{pfx}_{di}_{q}", tag="dAq")
            boundary = 0 if not rev else L - 1
            nc.vector.memset(
                dAq.rearrange("p (q f) -> p q f", q=NQ)[:, :, boundary:boundary + 1], 0)
            for j in range(NQ):
                n = q * NQ + j
                sc = st.A[:, di * DS + n:di * DS + n + 1]
                if not rev:
                    nc.scalar.activation(dAq[:, j * L + 1:(j + 1) * L],
                                         st.delta[di][:, 1:L], AF.Exp, scale=sc)
                else:
                    nc.scalar.activation(dAq[:, j * L:(j + 1) * L - 1],
                                         st.delta[di][:, 0:L - 1], AF.Exp, scale=sc)
            # dBu quad (vector, 2x bf16)
            dBuq = pp.spool.tile([128, QW], BF16, name=f"dBu_{pfx}_{di}_{q}", tag="dBuq")
            nc.vector.tensor_mul(dBuq.rearrange("p (q f) -> p q f", q=NQ),
                                 du_rep,
                                 Brep[:, ts(q, QW)].rearrange("p (q f) -> p q f", q=NQ))
            # merged segmented scan
            hq = pp.spool.tile([128, QW], BF16, name=f"h_{pfx}_{di}_{q}", tag="hq")
            if not rev:
                nc.vector.tensor_tensor_scan(hq, dAq, dBuq, 0.0, AOP.mult, AOP.add)
            else:
                nc.vector.tensor_tensor_scan(hq[:, ::-1], dAq[:, ::-1], dBuq[:, ::-1],
                                             0.0, AOP.mult, AOP.add)
            # hC mult (in place) + y-reduce matmuls
            nc.vector.tensor_mul(hq, hq, Crep[:, ts(q, QW)])
            for j in range(NQ):
                n = q * NQ + j
                nc.tensor.matmul(ps_y, pp.ident, hq[:, ts(j, L)],
                                 start=(n == 0), stop=False)
            yield

        # y += u*D via diag(D) matmul; evacuate PSUM on scalar; gate with silu(z)
        dgD = pp.wpool.tile([128, 128], BF16, name=f"dgD_{pfx}_{di}", tag="dg", bufs=5)
        nc.vector.tensor_scalar_mul(dgD, pp.ident, st.Dc[:, di:di + 1])
        nc.tensor.matmul(ps_y, dgD, st.u[di], start=False, stop=True)
        yD = pp.spool.tile([128, L], BF16, name=f"yD_{pfx}_{di}", tag="yD", bufs=1)
        nc.scalar.activation(yD, ps_y, AF.Copy)
        ygt = pp.bpool.tile([128, L], BF16, name=f"yg_{pfx}_{di}", tag=f"yg{di}", bufs=1)
        nc.vector.tensor_mul(ygt, yD, st.silu_z[di])
        st.yg[di] = ygt
        if op_accum is not None:
            ow, ps_op = op_accum
            g, kk = di // 4, di % 4
            for mi in range(NDM):
                w = ow[g].rearrange("p (a m) -> p a m", a=4)[:, kk, ts(mi, 128)]
                nc.tensor.matmul(ps_op[mi], w, ygt, start=(di == 0), stop=(di == NDI - 1))


def _load_ow(tc, pp, pfx, ins):
    """Load out_wT [1024, 512] as two [128, 4, 512] chunk tiles."""
    nc = tc.nc
    out_wT = ins[pfx + "_out_wT"]
    ow = []
    for g in range(2):
        ch = pp.wpool.tile([128, 4 * L], BF16, name=f"ow_{pfx}_{g}", tag="ow", bufs=2)
        nc.sync.dma_start(ch.rearrange("p (a m) -> p a m", a=4),
                          out_wT[g * 512:(g + 1) * 512, :].rearrange(
                              "(a p) m -> p a m", p=128))
        ow.append(ch)
    return ow


def _out_proj_mi(tc, pp, pfx, ins, xTb, ow):
    """out_proj with mi-outer loop; x-residual folded in as an identity MM."""
    nc = tc.nc
    st = pp.st[pfx]
    new_cur = []
    for mi in range(NDM):
        ps = pp.pwork.tile([128, L], FP32, name=f"ps_op_{pfx}_{mi}", tag="work")
        for ki in range(NDI):
            w = ow[ki // 4].rearrange("p (a m) -> p a m", a=4)[:, ki % 4, ts(mi, 128)]
            nc.tensor.matmul(ps, w, st.yg[ki], start=(ki == 0), stop=False)
        nc.tensor.matmul(ps, pp.ident, xTb[mi], start=False, stop=True)
        ncur = pp.bpool.tile([128, L], FP32, name=f"cur_{pfx}_{mi}", tag=f"cur{mi}", bufs=1)
        nc.scalar.activation(ncur, ps, AF.Copy)
        new_cur.append(ncur)
    return new_cur


def _layernorm(tc, pp, x_tiles, g_ap, b_ap, name, keep_all=True):
    """LN over the partition(feature) axis of transposed tiles, via PE colsums."""
    nc = tc.nc
    bpool, pwork = pp.bpool, pp.pwork
    ones = pp.cpool.tile([128, 1], BF16, name=f"ones_{name}", tag="ones")
    nc.vector.memset(ones, 1.0)
    ps_s = pwork.tile([1, L], FP32, name=f"ps_s_{name}", tag="work")
    ps_q = pwork.tile([1, L], FP32, name=f"ps_q_{name}", tag="work")
    for ki in range(NDM):
        xb = bpool.tile([128, L], BF16, name=f"xb_{name}_{ki}", tag="lnxb")
        nc.scalar.activation(xb, x_tiles[ki], AF.Copy)
        nc.tensor.matmul(ps_s, ones, xb, start=(ki == 0), stop=(ki == NDM - 1))
        sq = bpool.tile([128, L], BF16, name=f"sq_{name}_{ki}", tag="sq")
        nc.scalar.activation(sq, x_tiles[ki], AF.Square)
        nc.tensor.matmul(ps_q, ones, sq, start=(ki == 0), stop=(ki == NDM - 1))
    mean = bpool.tile([1, L], FP32, name=f"mean_{name}", tag="st1")
    nc.vector.tensor_scalar_mul(mean, ps_s, 1.0 / DM)
    msq = bpool.tile([1, L], FP32, name=f"msq_{name}", tag="st2")
    nc.vector.tensor_mul(msq, mean, mean)
    var = bpool.tile([1, L], FP32, name=f"var_{name}", tag="st3")
    nc.vector.scalar_tensor_tensor(var, ps_q, 1.0 / DM, msq, AOP.mult, AOP.subtract)
    sd = bpool.tile([1, L], FP32, name=f"sd_{name}", tag="st4")
    epsc = bpool.tile([1, 1], FP32, name=f"eps_{name}", tag="eps")
    nc.vector.memset(epsc, 1e-5)
    nc.scalar.activation(sd, var, AF.Sqrt, bias=epsc)
    istd = bpool.tile([1, L], FP32, name=f"istd_{name}", tag="st5")
    nc.vector.reciprocal(istd, sd)
    shift = bpool.tile([1, L], FP32, name=f"shift_{name}", tag="st6")
    nc.vector.tensor_mul(shift, mean, istd)
    nc.vector.tensor_scalar_mul(shift, shift, -1.0)
    istdb = bpool.tile([1, L], BF16, name=f"istdb_{name}", tag="st7")
    nc.vector.tensor_copy(istdb, istd)
    shiftb = bpool.tile([1, L], BF16, name=f"shiftb_{name}", tag="st8")
    nc.vector.tensor_copy(shiftb, shift)
    istd_r = bpool.tile([128, L], BF16, name=f"istd_r_{name}", tag="istd_r", bufs=1)
    shift_r = bpool.tile([128, L], BF16, name=f"shift_r_{name}", tag="shift_r", bufs=1)
    ps_b1 = pwork.tile([128, L], FP32, name=f"psb1_{name}", tag="work")
    nc.tensor.matmul(ps_b1, pp.ones_w, istdb, start=True, stop=True)
    nc.scalar.activation(istd_r, ps_b1, AF.Copy)
    ps_b2 = pwork.tile([128, L], FP32, name=f"psb2_{name}", tag="work")
    nc.tensor.matmul(ps_b2, pp.ones_w, shiftb, start=True, stop=True)
    nc.scalar.activation(shift_r, ps_b2, AF.Copy)
    gc = bpool.tile([128, NDM], FP32, name=f"g_{name}", tag=f"g_{name}")
    nc.sync.dma_start(gc, _mcols(g_ap, NDM))
    bc = bpool.tile([128, NDM], FP32, name=f"b_{name}", tag=f"b_{name}")
    nc.sync.dma_start(bc, _mcols(b_ap, NDM))
    out_tiles = []
    for ki in range(NDM):
        t1 = bpool.tile([128, L], FP32, name=f"t1_{name}_{ki}", tag="lnt1")
        nc.vector.tensor_mul(t1, x_tiles[ki], istd_r)
        nc.vector.tensor_add(t1, t1, shift_r)
        if keep_all:
            t3 = bpool.tile([128, L], BF16, name=f"t3_{name}_{ki}", tag=f"xTb{ki}")
        else:
            t3 = bpool.tile([128, L], FP32, name=f"t3_{name}_{ki}", tag="ln_out", bufs=2)
        nc.scalar.activation(t3, t1, AF.Identity, scale=gc[:, ki:ki + 1],
                             bias=bc[:, ki:ki + 1])
        out_tiles.append(t3)
    return out_tiles


def _drain(gen):
    for _ in gen:
        pass


def _kernel(tc, out_d, ins):
    nc = tc.nc
    pp = P()
    with (tc.tile_pool(name="const", bufs=1) as cpool,
          tc.tile_pool(name="big", bufs=1) as bpool,
          tc.tile_pool(name="wts", bufs=2) as wpool,
          tc.tile_pool(name="scan", bufs=2) as spool,
          tc.tile_pool(name="pwork", bufs=2, space="PSUM") as pwork,
          tc.tile_pool(name="pyacc", bufs=2, space="PSUM") as pyacc,
          tc.tile_pool(name="pacc4", bufs=4, space="PSUM") as pacc4):
        pp.cpool, pp.bpool, pp.wpool, pp.spool = cpool, bpool, wpool, spool
        pp.pwork, pp.pyacc, pp.pacc4 = pwork, pyacc, pacc4
        pp.st = {}

        pp.ident = cpool.tile([128, 128], BF16, name="ident", tag="ident")
        make_identity(nc, pp.ident)
        pp.ones_col = cpool.tile([128, 1], FP32, name="ones_col", tag="ones_col")
        nc.vector.memset(pp.ones_col, 1.0)
        pp.ones_w = cpool.tile([1, 128], BF16, name="ones_w", tag="ones_w")
        nc.vector.memset(pp.ones_w, 1.0)

        xTb = []
        for i in range(NDM):
            xtb = bpool.tile([128, L], BF16, name=f"xTb_{i}", tag=f"xTb{i}")
            nc.scalar.dma_start(xtb, ins["xbT16"][ts(i, 128), :])
            xTb.append(xtb)

        # ---- phase A: fwd front-end (uses the Silu table, ends in nat_log_exp)
        _drain(_frontend_units(tc, pp, ins, xTb, "f", False, use_silu_table=True))

        # ---- phase B: fwd scan block interleaved with rev front-end
        fe_r = _frontend_units(tc, pp, ins, xTb, "r", True, use_silu_table=False)
        for _ in _scan_units(tc, pp, "f", False, op_accum=None):
            next(fe_r, None)
        _drain(fe_r)

        # ---- phase C: fwd out_proj (mi-outer) + rev scan block with fused
        #      rev out_proj accumulation (ki-outer, pacc4)
        ow_f = _load_ow(tc, pp, "f", ins)
        cur1 = _out_proj_mi(tc, pp, "f", ins, xTb, ow_f)
        ow_r = _load_ow(tc, pp, "r", ins)
        ps_opr = [pacc4.tile([128, L], FP32, name=f"ps_opr_{mi}", tag="acc4")
                  for mi in range(NDM)]
        _drain(_scan_units(tc, pp, "r", True, op_accum=(ow_r, ps_opr)))
        cur2 = cur1
        for mi in range(NDM):
            nc.vector.scalar_tensor_tensor(cur2[mi], ps_opr[mi], 1.0, cur1[mi],
                                           AOP.mult, AOP.add)

        # ---- phase D: LN1, FFN, LN2
        x1 = _layernorm(tc, pp, cur2, ins["ln1_g"], ins["ln1_b"], "ln1")

        conv1_wT = ins["conv1_wT"]
        conv2_wT = ins["conv2_wT"]
        c1b = wpool.tile([128, NFF], FP32, name="c1b", tag="c1b", bufs=1)
        nc.sync.dma_start(c1b, _mcols(ins["conv1_b"], NFF))
        c2b = wpool.tile([128, NDM], FP32, name="c2b", tag="c2b", bufs=1)
        nc.sync.dma_start(c2b, _mcols(ins["conv2_b"], NDM))
        x1b = x1
        w1all = bpool.tile([128, DS * L], BF16, name="w1all", tag="Brep", bufs=1)
        for ki in range(NDM):
            nc.sync.dma_start(w1all[:, ki * DFF:(ki + 1) * DFF],
                              conv1_wT[ts(ki, 128), :])
        w2all = bpool.tile([128, DS * L], BF16, name="w2all", tag="Crep", bufs=1)
        nc.sync.dma_start(w2all.rearrange("p (a m) -> p a m", a=NFF),
                          conv2_wT.rearrange("(a p) m -> p a m", p=128))
        ps2 = [pacc4.tile([128, L], FP32, name=f"ps_ffn_{mi}", tag="acc4")
               for mi in range(NDM)]
        for ffi in range(NFF):
            ps1 = pwork.tile([128, L], FP32, name=f"ps_ff1_{ffi}", tag="work")
            for ki in range(NDM):
                w = w1all[:, ki * DFF + ffi * 128:ki * DFF + (ffi + 1) * 128]
                nc.tensor.matmul(ps1, w, x1b[ki], start=(ki == 0), stop=(ki == NDM - 1))
            y1 = bpool.tile([128, L], BF16, name=f"y1_{ffi}", tag="y1", bufs=2)
            nc.scalar.activation(y1, ps1, AF.Gelu, bias=c1b[:, ffi:ffi + 1])
            for mi in range(NDM):
                w2 = w2all[:, ffi * 512 + mi * 128:ffi * 512 + (mi + 1) * 128]
                nc.tensor.matmul(ps2[mi], w2, y1, start=(ffi == 0), stop=(ffi == NFF - 1))
        x2 = []
        for mi in range(NDM):
            t = bpool.tile([128, L], FP32, name=f"x2_{mi}", tag=f"cur{mi}")
            nc.vector.scalar_tensor_tensor(t, ps2[mi], 1.0, x1[mi], AOP.mult, AOP.add)
            nc.scalar.activation(t, t, AF.Identity, bias=c2b[:, mi:mi + 1])
            x2.append(t)

        out_t = _layernorm(tc, pp, x2, ins["ln2_g"], ins["ln2_b"], "ln2", keep_all=False)
        for mi in range(NDM):
            nc.scalar.dma_start(out_d[ts(mi, 128), :], out_t[mi])


@functools.lru_cache(maxsize=1)
def _build():
    nc = bacc.Bacc("TRN2", debug=False)
    ins = {"xbT": nc.dram_tensor("xbT", (DM, L), FP32, kind="ExternalInput").ap()}
    for name, shape in W_SHAPES.items():
        ins[name] = nc.dram_tensor(name, shape, FP32, kind="ExternalInput").ap()
    for name, shape in T_SHAPES.items():
        ins[name] = nc.dram_tensor(name, shape, BF16, kind="ExternalInput").ap()
    ins["xbT16"] = nc.dram_tensor("xbT16", (DM, L), BF16, kind="ExternalInput").ap()
    out_d = nc.dram_tensor("out", (DM, L), FP32, kind="ExternalOutput").ap()
    with tile.TileContext(nc) as tc:
        _kernel(tc, out_d, ins)
    nc.compile()
    return nc


def make_in_maps(inputs):
    import ml_dtypes
    bf = ml_dtypes.bfloat16
    shared = {}
    for name in W_SHAPES:
        shared[name] = np.ascontiguousarray(inputs[name], dtype=np.float32)
    for tname, sname in T_SOURCES.items():
        shared[tname] = np.ascontiguousarray(
            np.asarray(inputs[sname], dtype=np.float32).T).astype(bf)
    in_maps = []
    for c in range(NB):
        m = dict(shared)
        xt = np.ascontiguousarray(np.asarray(inputs["x"][c], dtype=np.float32).T)
        m["xbT"] = xt
        m["xbT16"] = xt.astype(bf)
        in_maps.append(m)
    return in_maps


def kernel(**inputs):
    nc = _build()
    res = run_bass_kernel_spmd(nc, make_in_maps(inputs), list(range(NB)))
    return np.stack([res.results[c]["out"].T for c in range(NB)]).astype(np.float32)
